# revision 23
# baseline (speedup 1.0000x reference)
"""Distributed Trainium2 kernel for nn_Criterion_35012573397697.

Proxy-NCA-style loss: mean_b[ d(x_b, p_{y_b}) + logsumexp_{c != y_b}(-d(x_b, p_c)) ]
with x = 3*l2norm(batch), p = 3*l2norm(proxies), d = squared euclidean.

v2 strategy (8 NeuronCores, classes sharded, SH=12544/core):
  - d(x,p) = 18 - 2*x.p on the 3-sphere; only dot products needed.
  - Per chunk of 8 c-tiles: DMA praw; pn2 via one fused tensor_tensor_reduce
    pass (DVE); 3/sqrt via bit-trick rsqrt (DVE); scale+f32->bf16 convert
    split DVE/Pool; PE transpose; PSUM->SBUF copy on Pool.
  - Matmul groups of 12 tiles (3x512 cols) per b-tile; ScalarE computes
    exp(2*s - 18) with fused row-sum (accum_out) directly from PSUM.
  - Two pipelined AllReduce(add) of [128,4] partial sums: AR1 mid-stream
    (absorbs collective warm-up + inter-core skew under remaining compute),
    AR2 at the end. Ln table preloaded during the AR2 wait; exp(-pos_dist)
    computed early from host-gathered proxies[labels] rows (indexing only).
"""

import math

import numpy as np
import ml_dtypes

import concourse.bass as bass
import concourse.bacc as bacc
import concourse.mybir as mybir
import concourse.tile as tile
from concourse.bass_utils import run_bass_kernel_spmd

N_CORES = 8
B = 512
D = 128
C = 100000
SH = 12544           # padded shard size per core (98 tiles)
NT = SH // 128       # 98 c-tiles of 128
BT = B // 128        # 4 b-tiles
PAD_ROWS = N_CORES * SH - C   # 352 zero rows in total
PAD_CORR = PAD_ROWS * math.exp(-18.0)

F32 = mybir.dt.float32
BF16 = mybir.dt.bfloat16
AX = mybir.AxisListType
OP = mybir.AluOpType
AF = mybir.ActivationFunctionType

_CACHE = {}

# chunking of the 98 c-tiles for DMA + norm pipeline
CHUNKS = [8] * 12 + [2]
# matmul/exp groups (in tiles) per b-tile; each <= 12 tiles = 1536 psum cols
GROUPS = [12] * 8 + [2]
AR1_GROUPS = 6        # groups 0..5 (72 tiles) go into AllReduce #1


def _rsqrt_dve(nc, pool, dst, src, n, scale=1.0):
    """dst = scale / sqrt(src) via bit trick + two Newton steps (DVE only)."""
    I32 = mybir.dt.int32
    v = pool.tile([128, n], F32, tag="rsq_v")
    nc.vector.tensor_scalar(v[:], src, 1e-12, None, OP.max)
    src = v[:]
    h = pool.tile([128, n], I32, tag="rsq_h")
    nc.vector.tensor_scalar(h[:], src.bitcast(I32), 1, None,
                            OP.logical_shift_right)
    y0 = pool.tile([128, n], I32, tag="rsq_y0")
    nc.vector.tensor_scalar(y0[:], h[:], -1, 0x5F3759DF, OP.mult, OP.add)
    y0f = y0[:].bitcast(F32)
    t = pool.tile([128, n], F32, tag="rsq_t")
    y1 = pool.tile([128, n], F32, tag="rsq_y1")
    nc.vector.tensor_tensor(t[:], y0f, y0f, OP.mult)        # y0^2
    nc.vector.tensor_tensor(t[:], t[:], src, OP.mult)       # v*y0^2
    nc.vector.tensor_scalar(t[:], t[:], -0.5, 1.5, OP.mult, OP.add)
    nc.vector.tensor_tensor(y1[:], y0f, t[:], OP.mult)      # Newton 1
    nc.vector.tensor_tensor(t[:], y1[:], y1[:], OP.mult)    # y1^2
    nc.vector.tensor_tensor(t[:], t[:], src, OP.mult)       # v*y1^2
    nc.vector.tensor_scalar(t[:], t[:], -0.5 * scale, 1.5 * scale,
                            OP.mult, OP.add)
    nc.vector.tensor_tensor(dst, y1[:], t[:], OP.mult)      # Newton 2


def _rsqrt1_dve(nc, pool, dst, src, n, scale=1.0):
    """dst = scale / sqrt(src) via bit trick + one Newton step (DVE only)."""
    I32 = mybir.dt.int32
    v = pool.tile([128, n], F32, tag="rsq_v")
    nc.vector.tensor_scalar(v[:], src, 1e-12, None, OP.max)
    h = pool.tile([128, n], I32, tag="rsq_h")
    nc.vector.tensor_scalar(h[:], v[:].bitcast(I32), 1, None,
                            OP.logical_shift_right)
    y0 = pool.tile([128, n], I32, tag="rsq_y0")
    nc.vector.tensor_scalar(y0[:], h[:], -1, 0x5F3759DF, OP.mult, OP.add)
    y0f = y0[:].bitcast(F32)
    t = pool.tile([128, n], F32, tag="rsq_t")
    nc.vector.tensor_tensor(t[:], y0f, y0f, OP.mult)        # y0^2
    nc.vector.tensor_tensor(t[:], t[:], v[:], OP.mult)      # v*y0^2
    nc.vector.tensor_scalar(t[:], t[:], -0.5 * scale, 1.5 * scale,
                            OP.mult, OP.add)
    nc.vector.tensor_tensor(dst, y0f, t[:], OP.mult)        # Newton 1


def build_graph():
    nc = bacc.Bacc("TRN2", target_bir_lowering=False, debug=False,
                   num_devices=N_CORES)
    p_ext = nc.dram_tensor("pshard", [SH, D], F32, kind="ExternalInput").ap()
    b_ext = nc.dram_tensor("batch", [B, D], F32, kind="ExternalInput").ap()
    sel_ext = nc.dram_tensor("psel", [B, D], F32, kind="ExternalInput").ap()
    id_ext = nc.dram_tensor("ident", [128, 128], BF16, kind="ExternalInput").ap()
    out_ext = nc.dram_tensor("out", [1, 1], F32, kind="ExternalOutput").ap()

    NG = len(GROUPS)
    g_tile_lo = [sum(GROUPS[:i]) for i in range(NG)]

    with tile.TileContext(nc) as tc:
        with tc.tile_pool(name="dram", bufs=1, space="DRAM") as dram, \
             tc.tile_pool(name="big", bufs=1) as bigp, \
             tc.tile_pool(name="sb", bufs=2) as pool, \
             tc.tile_pool(name="ps", bufs=2, space="PSUM") as psp, \
             tc.tile_pool(name="pst", bufs=2, space="PSUM") as pst:

            # early tiny AllReduce: absorbs the first-collective warm-up
            # while the DMA stream runs; result folded in as exact zero.
            dag_in = dram.tile([1, 16], F32)
            dag_out = dram.tile([1, 16], F32)
            z16 = bigp.tile([1, 16], F32)
            nc.vector.memset(z16[:], 0.0)
            dagj = bigp.tile([1, 1], F32)
            nc.sync.dma_start(dag_in[:], z16[:])
            nc.gpsimd.collective_compute(
                "AllReduce", OP.add,
                replica_groups=[list(range(N_CORES))],
                ins=[dag_in.opt()], outs=[dag_out.opt()],
            )
            nc.sync.dma_start(dagj[:], dag_out[0:1, 0:1])

            # ---------- loads: chunk0 first, then x-side inputs ----------
            praw = bigp.tile([128, NT, 128], F32)       # [c%128, t, d]
            psrc = p_ext.rearrange("(t p) d -> p t d", p=128)
            ident = bigp.tile([128, 128], BF16)
            xb = bigp.tile([128, BT, 128], F32)         # [b%128, bt, d]
            selb = bigp.tile([128, BT, 128], F32)
            clo = 0
            for ci, ck in enumerate(CHUNKS):
                nc.sync.dma_start(praw[:, clo:clo + ck, :],
                                  psrc[:, clo:clo + ck, :])
                clo += ck
                if ci == 0:
                    nc.sync.dma_start(xb[:],
                                      b_ext.rearrange("(t p) d -> p t d", p=128))
                    nc.sync.dma_start(ident[:], id_ext[:])
                elif ci == 1:
                    nc.sync.dma_start(
                        selb[:], sel_ext.rearrange("(t p) d -> p t d", p=128))

            bias18 = bigp.tile([128, 1], F32)
            nc.vector.memset(bias18[:], -18.0)

            # ---------- x / psel prep (tiny, fused square+rowsum) ----------
            n2 = bigp.tile([128, 2 * BT], F32)
            for t in range(BT):
                xscr = pool.tile([128, 128], BF16, tag="sqscr")
                nc.vector.scalar_tensor_tensor(
                    xscr[:], xb[:, t, :], 1.0, xb[:, t, :],
                    op0=OP.mult, op1=OP.mult, accum_out=n2[:, t:t + 1])
            rn = bigp.tile([128, 2 * BT], F32)   # 1/sqrt(n2)
            posdot = bigp.tile([128, BT], F32)
            for t in range(BT):
                xscr = pool.tile([128, 128], BF16, tag="sqscr")
                nc.vector.scalar_tensor_tensor(
                    xscr[:], selb[:, t, :], 1.0, selb[:, t, :],
                    op0=OP.mult, op1=OP.mult,
                    accum_out=n2[:, BT + t:BT + t + 1])
            _rsqrt_dve(nc, pool, rn[:], n2[:], 2 * BT)
            for t in range(BT):
                xscr = pool.tile([128, 128], BF16, tag="sqscr")
                nc.vector.scalar_tensor_tensor(
                    xscr[:], xb[:, t, :], 1.0, selb[:, t, :],
                    op0=OP.mult, op1=OP.mult,
                    accum_out=posdot[:, t:t + 1])
            posd = bigp.tile([128, BT], F32)
            tmp4 = pool.tile([128, BT], F32, tag="smallscr")
            nc.vector.tensor_tensor(tmp4[:], posdot[:], rn[:, 0:BT], OP.mult)
            nc.vector.tensor_tensor(tmp4[:], tmp4[:], rn[:, BT:2 * BT], OP.mult)
            nc.vector.tensor_scalar(posd[:], tmp4[:], -18.0, 18.0, OP.mult,
                                    OP.add)
            # exp(-pos_dist) early (Exp table is resident during the stream)
            npos = bigp.tile([128, BT], F32)
            nc.scalar.activation(npos[:], posd[:], AF.Exp, scale=-1.0)

            # x-hat bf16 + transpose via PE
            xscale3 = bigp.tile([128, BT], F32)
            nc.vector.tensor_scalar_mul(xscale3[:], rn[:, 0:BT], 3.0)
            xhat = bigp.tile([128, BT, 128], BF16)
            for t in range(BT):
                nc.vector.tensor_scalar_mul(xhat[:, t, :], xb[:, t, :],
                                            xscale3[:, t:t + 1])
            xT = bigp.tile([128, BT, 128], BF16)
            xps = pst.tile([128, 4 * 128], BF16, tag="tp")
            for t in range(BT):
                nc.tensor.transpose(xps[:, t * 128:(t + 1) * 128],
                                    xhat[:, t, :], ident[:])
            nc.vector.tensor_copy(
                xT[:], xps[:, 0:BT * 128].rearrange("p (t d) -> p t d", t=BT))

            # ---------- proxy pipeline ----------
            pn2 = bigp.tile([128, NT], F32)
            pscale3 = bigp.tile([128, NT], F32)
            pbf = bigp.tile([128, NT, 128], BF16)
            pT = bigp.tile([128, NT, 128], BF16)   # [d, t, c%128]
            pTf = pT[:].rearrange("p t c -> p (t c)")
            partials = bigp.tile([128, BT * NG], F32)

            def issue_group(bt, g):
                ntg = GROUPS[g]
                lo = g_tile_lo[g]
                ncols = ntg * 128
                sp = psp.tile([128, 1536], F32, tag="sc")
                for j in range(0, ntg, 4):
                    w = min(4, ntg - j) * 128
                    nc.tensor.matmul(
                        sp[:, j * 128:j * 128 + w],
                        xT[:, bt, :],
                        pTf[:, (lo + j) * 128:(lo + j) * 128 + w],
                        start=True, stop=True)
                ej = pool.tile([128, 1536], BF16, tag="ejunk")
                nc.scalar.activation(
                    ej[:, 0:ncols], sp[:, 0:ncols], AF.Exp,
                    bias=bias18[:, 0:1], scale=2.0,
                    accum_out=partials[:, bt * NG + g:bt * NG + g + 1])

            issued = set()
            # process chunk PAIRS: batched rsqrt + Pool scale, 8-tile
            # transpose/copy granularity
            PAIRS = [(0, 16), (16, 32), (32, 48), (48, 64), (64, 80),
                     (80, 96), (96, 98)]
            for lo, hi in PAIRS:
                ck = hi - lo
                # pn2 via fused square + row-sum (one DVE pass per tile;
                # Pool is avoided for bulk work: it halves DVE throughput
                # through the shared SBUF ports)
                for t in range(lo, hi):
                    psq = pool.tile([128, 128], BF16, tag="psq")
                    nc.vector.scalar_tensor_tensor(
                        psq[:], praw[:, t, :], 1.0, praw[:, t, :],
                        op0=OP.mult, op1=OP.mult,
                        accum_out=pn2[:, t:t + 1])
                # 3/sqrt(pn2), one Newton step, batched over the pair
                _rsqrt1_dve(nc, pool, pscale3[:, lo:hi], pn2[:, lo:hi], ck,
                            scale=3.0)
                # scale + f32->bf16 convert: mostly DVE, a slice on Pool
                mid = lo + (2 * ck) // 8
                if mid > lo:
                    nc.gpsimd.tensor_tensor(
                        pbf[:, lo:mid, :], praw[:, lo:mid, :],
                        pscale3[:, lo:mid, None].to_broadcast(
                            (128, mid - lo, 128)),
                        OP.mult)
                for t in range(mid, hi):
                    nc.vector.tensor_scalar_mul(pbf[:, t, :], praw[:, t, :],
                                                pscale3[:, t:t + 1])
                # PE transpose + PSUM->SBUF copy on DVE (f32-bitcast view
                # halves the column count of the raw bit copy)
                for s in range(lo, hi, 8):
                    e = min(s + 8, hi)
                    tp = pst.tile([128, 8 * 128], BF16, tag="tp")
                    for j in range(e - s):
                        nc.tensor.transpose(
                            tp[:, j * 128:(j + 1) * 128],
                            pbf[:, s + j, :], ident[:])
                    nc.vector.tensor_copy(
                        pT[:, s:e, :].bitcast(F32),
                        tp[:, 0:(e - s) * 128].bitcast(F32).rearrange(
                            "p (t d) -> p t d", t=e - s))
                # issue any groups whose tiles are now complete
                for g in range(NG):
                    if (g not in issued
                            and g_tile_lo[g] + GROUPS[g] <= hi):
                        issued.add(g)
                        for bt in range(BT):
                            issue_group(bt, g)

            # ---------- single AllReduce of the partial sums ----------
            par3 = partials[:].rearrange("p (t g) -> p t g", t=BT)
            s1 = bigp.tile([128, BT], F32)
            nc.vector.tensor_reduce(s1[:], par3[:],
                                    axis=AX.X, op=OP.add)
            ar1_in = dram.tile([128, BT], F32)
            ar1_out = dram.tile([128, BT], F32)
            nc.sync.dma_start(ar1_in[:], s1[:])
            nc.gpsimd.collective_compute(
                "AllReduce", OP.add,
                replica_groups=[list(range(N_CORES))],
                ins=[ar1_in.opt()], outs=[ar1_out.opt()],
            )
            g1 = bigp.tile([128, BT], F32)
            nc.sync.dma_start(g1[:], ar1_out[:])

            # preload the Ln activation table while AR2 is in flight; the
            # input is the last exp group's accumulator so the scheduler
            # cannot hoist this (and its table swap) before the exp stream.
            lnj = pool.tile([1, 1], F32, tag="fin")
            nc.scalar.activation(lnj[:],
                                 partials[0:1, BT * NG - 1:BT * NG],
                                 AF.Ln)

            # ---------- final scalar (identical on every core) ----------
            sneg = pool.tile([128, BT], F32, tag="fin")
            nc.vector.tensor_scalar(sneg[:], g1[:], -float(PAD_CORR),
                                    None, OP.add)
            nc.vector.tensor_tensor(sneg[:], sneg[:], npos[:], OP.subtract)
            lse = pool.tile([128, BT], F32, tag="fin")
            nc.scalar.activation(lse[:], sneg[:], AF.Ln)
            perb = pool.tile([128, BT], F32, tag="fin")
            nc.vector.tensor_tensor(perb[:], posd[:], lse[:], OP.add)
            csum = pool.tile([128, 1], F32, tag="fin")
            nc.vector.tensor_reduce(csum[:], perb[:], axis=AX.X, op=OP.add)
            nc.vector.tensor_tensor(csum[0:1, 0:1], csum[0:1, 0:1],
                                    dagj[:], OP.add)
            ones = pool.tile([128, 1], F32, tag="fin")
            nc.vector.memset(ones[:], 1.0)
            lps = psp.tile([1, 1], F32, tag="sc")
            nc.tensor.matmul(lps[:], ones[:], csum[:], start=True, stop=True)
            res = pool.tile([1, 1], F32, tag="fin")
            nc.scalar.activation(res[:], lps[:], AF.Copy, scale=1.0 / B)
            nc.sync.dma_start(out_ext[:], res[:])

    nc.compile()
    return nc


def make_in_maps(batch, labels, proxies):
    batch = np.ascontiguousarray(batch, dtype=np.float32)
    labels = np.asarray(labels).astype(np.int64)
    proxies = np.ascontiguousarray(proxies, dtype=np.float32)
    psel = np.ascontiguousarray(proxies[labels])        # indexing only
    ident = np.eye(128, dtype=np.float32).astype(ml_dtypes.bfloat16)
    ppad = np.zeros((N_CORES * SH, D), dtype=np.float32)
    ppad[:C] = proxies
    in_maps = []
    for i in range(N_CORES):
        in_maps.append({
            "pshard": np.ascontiguousarray(ppad[i * SH:(i + 1) * SH]),
            "batch": batch,
            "psel": psel,
            "ident": ident,
        })
    return in_maps


def _get_nc():
    if "nc" not in _CACHE:
        _CACHE["nc"] = build_graph()
    return _CACHE["nc"]


def kernel(batch, labels, proxies):
    nc = _get_nc()
    in_maps = make_in_maps(batch, labels, proxies)
    try:
        res = run_bass_kernel_spmd(nc, in_maps, core_ids=list(range(N_CORES)))
    except Exception:
        # transient device hiccup: retry once
        res = run_bass_kernel_spmd(nc, in_maps, core_ids=list(range(N_CORES)))
    return np.float32(res.results[0]["out"][0, 0])


if __name__ == "__main__":
    rng = np.random.default_rng(0)
    batch = rng.standard_normal((B, D)).astype(np.float32)
    labels = rng.integers(0, C, B).astype(np.int64)
    proxies = (rng.standard_normal((C, D)).astype(np.float32) / 8)
    out = kernel(batch=batch, labels=labels, proxies=proxies)
    print("loss:", out)
